# revision 1
# baseline (speedup 1.0000x reference)
"""Trainium2 Bass kernel: MeanHinAggregator (GNN message passing).

Reference computation (per batch-head element bh):
    z_r  = mean_n(x_neigh_r[bh, n, :]) @ w_neigh_r          (r = 0, 1)
    out  = relu(concat(x_self[bh] @ w_self, (z0 + z1) / 2) + b)

Strategy (pure data parallel over 8 NeuronCores, batch axis sharded):
  * Per core: B_shard=128, H=10 -> 1280 rows, processed in 10 groups of 128.
  * Neighbour tiles are DMA'd in natural layout [128 bh-part, (n f) free]
    (16 KiB contiguous per partition -> full DMA bandwidth).
  * The mean-over-neighbours reduction is split between engines: two in-place
    strided adds on the Vector engine fold the 32 neighbour slices to 8 (fp32
    matmuls cost a double LDWEIGHTS+MATMUL pass on TRN2, so DVE folding is
    ~4x cheaper per element than PE matmuls), then 8 accumulating matmuls
    with lhsT = x-slice, rhs = identity finish the sum:
        psum[f, bh] += sum_k x[k, n*128+f] * I[k, bh]  ==  x[bh, n, f]
    i.e. each matmul transposes one folded slice into PSUM while the PSUM
    accumulation performs the remaining sum over n.  This yields the
    neighbour sums directly in the [f, bh] layout the projection matmuls
    need as lhsT (the TensorEngine always contracts over the partition axis).
  * Projection: out[bh, d] = lhsT(sumT).T @ w.  The 1/(N*NR) mean scaling is
    folded into host-prescaled copies of w_neigh_*.  Bias is added with a
    K=1 matmul (lhsT = ones row, rhs = bias row) accumulating into PSUM.
  * Final ReLU on the Scalar engine (PSUM -> SBUF), then DMA out.
"""

import numpy as np

import concourse.bacc as bacc
import concourse.bass as bass
import concourse.tile as tile
from concourse import bass_utils, mybir
from concourse._compat import with_exitstack

B, H, N, F = 1024, 10, 32, 128
HALF = 128
D = 2 * HALF
NR = 2
NCORES = 8
BSH = B // NCORES        # 128 batch rows per core
BH = BSH * H             # 1280 (bh rows per core)
GROUP = 128              # bh rows per group
F32 = mybir.dt.float32


@with_exitstack
def _tile_kernel(ctx, tc, outs, ins, ngroups):
    nc = tc.nc
    xn0, xn1, xs, w_s, w0, w1, bvec, ident_d = ins
    (out_d,) = outs

    const = ctx.enter_context(tc.tile_pool(name="const", bufs=1))
    xpool = ctx.enter_context(tc.tile_pool(name="xp", bufs=4))
    spool = ctx.enter_context(tc.tile_pool(name="sp", bufs=3))
    opool = ctx.enter_context(tc.tile_pool(name="op", bufs=3))
    ppool = ctx.enter_context(tc.tile_pool(name="ps", bufs=2, space="PSUM"))
    pout = ctx.enter_context(tc.tile_pool(name="po", bufs=2, space="PSUM"))

    def issue_loads(g):
        """Issue the group's input DMAs: the two 2 MiB neighbour blocks on
        alternating HWDGE rings (SP / ACT) so their latencies overlap."""
        r = slice(g * GROUP, (g + 1) * GROUP)
        t0 = xpool.tile([128, N * F], F32, tag="t0")
        nc.sync.dma_start(t0[:], xn0[r, :])
        t1 = xpool.tile([128, N * F], F32, tag="t1")
        nc.scalar.dma_start(t1[:], xn1[r, :])
        ts = spool.tile([128, F], F32, tag="ts")
        nc.sync.dma_start(ts[:], xs[r, :])
        return t0, t1, ts

    pending = issue_loads(0)

    ident = const.tile([128, 128], F32, tag="ident")
    nc.sync.dma_start(ident[:], ident_d[:])
    wS_t = const.tile([128, HALF], F32, tag="wS")
    nc.sync.dma_start(wS_t[:], w_s[:])
    w0_t = const.tile([128, HALF], F32, tag="w0")
    nc.sync.dma_start(w0_t[:], w0[:])
    w1_t = const.tile([128, HALF], F32, tag="w1")
    nc.sync.dma_start(w1_t[:], w1[:])
    b_t = const.tile([1, D], F32, tag="b")
    nc.sync.dma_start(b_t[:], bvec[:])
    ones_t = const.tile([1, 128], F32, tag="ones")
    nc.vector.memset(ones_t[:], 1.0)

    for g in range(ngroups):
        r = slice(g * GROUP, (g + 1) * GROUP)
        t0, t1, ts = pending
        if g + 1 < ngroups:
            pending = issue_loads(g + 1)

        # Fold all 32 neighbour slices into one with five in-place strided
        # adds on the Vector engine: fp32 matmuls pay a double
        # LDWEIGHTS+MATMUL pass on the PE (~470 ns each), so folding on DVE
        # is ~4x cheaper per element and shortens the post-DMA critical
        # chain.  (Measured: NFOLD=4/8 variants that shift fold levels back
        # to PE matmuls are 5-8 us slower end-to-end.)
        # After folding, t[:, 0:F] holds the full sum over the 32 neighbours.
        for t in (t0, t1):
            for lv in (16, 8, 4, 2, 1):
                nc.vector.tensor_add(t[:, 0:lv * F], t[:, 0:lv * F],
                                     t[:, lv * F:2 * lv * F])

        # One PSUM tile holds all three transposed operands side by side:
        # pacc[:, 0:128] = sum_n x_n0 (as [f, bh]), [:, 128:256] = sum_n x_n1,
        # [:, 256:384] = x_self.  Each is a single transposing matmul
        # (lhsT = data, rhs = identity -> out[f, bh] = data[bh, f]).
        pacc = ppool.tile([128, 3 * 128], F32, tag="pacc")
        nc.tensor.matmul(pacc[:, 0:128], t0[:, 0:F], ident[:],
                         start=True, stop=True)
        nc.tensor.matmul(pacc[:, 128:256], t1[:, 0:F], ident[:],
                         start=True, stop=True)
        nc.tensor.matmul(pacc[:, 256:384], ts[:], ident[:],
                         start=True, stop=True)

        # PSUM -> SBUF in one copy (matmul lhsT must live in SBUF)
        sacc = spool.tile([128, 3 * 128], F32, tag="sacc")
        nc.any.tensor_copy(sacc[:], pacc[:])

        # Projection: out[bh, d]; bias broadcast via K=1 matmuls.
        po = pout.tile([128, D], F32, tag="po")
        nc.tensor.matmul(po[:, 0:HALF], sacc[:, 256:384], wS_t[:],
                         start=True, stop=False)
        nc.tensor.matmul(po[:, 0:HALF], ones_t[:], b_t[:, 0:HALF],
                         start=False, stop=True)
        nc.tensor.matmul(po[:, HALF:D], sacc[:, 0:128], w0_t[:],
                         start=True, stop=False)
        nc.tensor.matmul(po[:, HALF:D], sacc[:, 128:256], w1_t[:],
                         start=False, stop=False)
        nc.tensor.matmul(po[:, HALF:D], ones_t[:], b_t[:, HALF:D],
                         start=False, stop=True)

        ob = opool.tile([128, D], F32, tag="ob")
        nc.scalar.activation(ob[:], po[:], mybir.ActivationFunctionType.Relu)
        nc.sync.dma_start(out_d[r, :], ob[:])


def build_nc(ngroups=BH // GROUP):
    bh = ngroups * GROUP
    nc = bacc.Bacc("TRN2", target_bir_lowering=False, debug=False)
    xn0 = nc.dram_tensor("xn0", [bh, N * F], F32, kind="ExternalInput")
    xn1 = nc.dram_tensor("xn1", [bh, N * F], F32, kind="ExternalInput")
    xs = nc.dram_tensor("xs", [bh, F], F32, kind="ExternalInput")
    w_s = nc.dram_tensor("w_s", [F, HALF], F32, kind="ExternalInput")
    w0 = nc.dram_tensor("w0", [F, HALF], F32, kind="ExternalInput")
    w1 = nc.dram_tensor("w1", [F, HALF], F32, kind="ExternalInput")
    bvec = nc.dram_tensor("bvec", [1, D], F32, kind="ExternalInput")
    ident_d = nc.dram_tensor("ident", [128, 128], F32, kind="ExternalInput")
    out = nc.dram_tensor("out", [bh, D], F32, kind="ExternalOutput")

    ins = [t.ap() for t in (xn0, xn1, xs, w_s, w0, w1, bvec, ident_d)]
    with tile.TileContext(nc) as tc:
        _tile_kernel(tc, [out.ap()], ins, ngroups)
    nc.compile()
    return nc


def make_in_maps(x_self, x_neigh_0, x_neigh_1, w_self, w_neigh_0, w_neigh_1, b):
    """Shard full inputs into per-core input maps (batch axis, 8 ways)."""
    x_self = np.ascontiguousarray(np.asarray(x_self, dtype=np.float32))
    x_neigh_0 = np.ascontiguousarray(np.asarray(x_neigh_0, dtype=np.float32))
    x_neigh_1 = np.ascontiguousarray(np.asarray(x_neigh_1, dtype=np.float32))
    scale = np.float32(1.0 / (N * NR))
    w_s = np.ascontiguousarray(np.asarray(w_self, dtype=np.float32))
    w0 = np.ascontiguousarray(np.asarray(w_neigh_0, dtype=np.float32) * scale)
    w1 = np.ascontiguousarray(np.asarray(w_neigh_1, dtype=np.float32) * scale)
    bvec = np.ascontiguousarray(np.asarray(b, dtype=np.float32).reshape(1, D))
    ident = np.eye(128, dtype=np.float32)

    in_maps = []
    for c in range(NCORES):
        bs = slice(c * BSH, (c + 1) * BSH)
        in_maps.append({
            "xn0": np.ascontiguousarray(x_neigh_0[bs].reshape(BH, N * F)),
            "xn1": np.ascontiguousarray(x_neigh_1[bs].reshape(BH, N * F)),
            "xs": np.ascontiguousarray(x_self[bs].reshape(BH, F)),
            "w_s": w_s, "w0": w0, "w1": w1, "bvec": bvec, "ident": ident,
        })
    return in_maps


_NC_CACHE = None


def kernel(x_self, x_neigh_0, x_neigh_1, w_self, w_neigh_0, w_neigh_1, b):
    global _NC_CACHE
    if _NC_CACHE is None:
        _NC_CACHE = build_nc()
    in_maps = make_in_maps(x_self, x_neigh_0, x_neigh_1,
                           w_self, w_neigh_0, w_neigh_1, b)
    res = bass_utils.run_bass_kernel_spmd(
        _NC_CACHE, in_maps, core_ids=list(range(NCORES)))
    out = np.concatenate([r["out"] for r in res.results], axis=0)
    return out.reshape(B, H, D)



# revision 2
# speedup vs baseline: 1.8453x; 1.8453x over previous
"""Trainium2 Bass kernel: MeanHinAggregator (GNN message passing).

Reference computation (per batch-head element bh):
    z_r  = mean_n(x_neigh_r[bh, n, :]) @ w_neigh_r          (r = 0, 1)
    out  = relu(concat(x_self[bh] @ w_self, (z0 + z1) / 2) + b)

Strategy (pure data parallel over 8 NeuronCores, batch axis sharded):
  * The 2e-2 relative-error budget admits a bf16 datapath.  All activations
    and weights are cast to bf16 on the host during the shard step, halving
    the HBM traffic that dominates this memory-bound problem (per core:
    2 x 10.5 MiB neighbour reads instead of 2 x 21 MiB).
  * Per core: B_shard=128, H=10 -> 1280 rows, processed in 10 groups of 128.
    Neighbour tiles are DMA'd in natural layout [128 bh-part, (n f) free]
    (8 KiB contiguous per partition at full DMA bandwidth); t0 rides the
    SP HWDGE ring, t1 + output stores ride the ACT ring, with a 3-group
    lookahead so both rings always have queued work.
  * Mean over the 32 neighbour slices: four in-place bf16 strided adds on
    the Vector engine (bf16 tensor_tensor hits the 2x_1P DVE perf mode)
    fold 32 slices to 2, then two accumulating transposing matmuls
    (lhsT = slice, rhs = identity) finish the sum while transposing into
    the [f, bh] layout the projection needs as lhsT.  Keeping the last
    fold level on the PE shortens the post-last-DMA tail, which is pure
    serial latency.
  * Projection: out[bh, d] = sumT.T @ w with the 1/(N*NR) scaling folded
    into host-prescaled bf16 copies of w_neigh_*.  Bias is added with a
    K=1 matmul (lhsT = ones row, rhs = bias row) accumulating into PSUM.
    PSUM -> SBUF copies are split per 128-column block on the Scalar
    engine so early projection matmuls don't wait on the last transpose.
  * Final ReLU on the Scalar engine produces fp32 (PSUM -> SBUF), then the
    store is issued from the Scalar engine so it dispatches right after
    ReLU with no cross-engine stall.
"""

import numpy as np
import ml_dtypes

import concourse.bacc as bacc
import concourse.bass as bass
import concourse.tile as tile
from concourse import bass_utils, mybir
from concourse._compat import with_exitstack

B, H, N, F = 1024, 10, 32, 128
HALF = 128
D = 2 * HALF
NR = 2
NCORES = 8
BSH = B // NCORES        # 128 batch rows per core
BH = BSH * H             # 1280 (bh rows per core)
GROUP = 128              # bh rows per group
LOOKAHEAD = 3            # groups of DMA prefetch beyond the current one
F32 = mybir.dt.float32
BF16 = mybir.dt.bfloat16
BF16NP = np.dtype(ml_dtypes.bfloat16)
RELU = mybir.ActivationFunctionType.Relu
COPY = mybir.ActivationFunctionType.Copy


@with_exitstack
def _tile_kernel(ctx, tc, outs, ins, ngroups):
    nc = tc.nc
    xn0, xn1, xs, w_s, w0, w1, bvec, ident_d, ones_d = ins
    (out_d,) = outs

    const = ctx.enter_context(tc.tile_pool(name="const", bufs=1))
    xpool = ctx.enter_context(tc.tile_pool(name="xp", bufs=LOOKAHEAD + 1))
    spool = ctx.enter_context(tc.tile_pool(name="sp", bufs=3))
    opool = ctx.enter_context(tc.tile_pool(name="op", bufs=3))
    ppool = ctx.enter_context(tc.tile_pool(name="ps", bufs=2, space="PSUM"))
    pout = ctx.enter_context(tc.tile_pool(name="po", bufs=2, space="PSUM"))

    def issue_loads(g):
        """t0 + x_self on the SP ring, t1 on the ACT ring (stores join ACT)."""
        r = slice(g * GROUP, (g + 1) * GROUP)
        t0 = xpool.tile([128, N * F], BF16, tag="t0")
        nc.sync.dma_start(t0[:], xn0[r, :])
        t1 = xpool.tile([128, N * F], BF16, tag="t1")
        nc.scalar.dma_start(t1[:], xn1[r, :])
        ts = xpool.tile([128, F], BF16, tag="ts")
        nc.sync.dma_start(ts[:], xs[r, :])
        return t0, t1, ts

    pending = [issue_loads(0)]

    ident = const.tile([128, 128], BF16, tag="ident")
    nc.sync.dma_start(ident[:], ident_d[:])
    wS_t = const.tile([128, HALF], BF16, tag="wS")
    nc.sync.dma_start(wS_t[:], w_s[:])
    w0_t = const.tile([128, HALF], BF16, tag="w0")
    nc.sync.dma_start(w0_t[:], w0[:])
    w1_t = const.tile([128, HALF], BF16, tag="w1")
    nc.sync.dma_start(w1_t[:], w1[:])
    b_t = const.tile([1, D], BF16, tag="b")
    nc.sync.dma_start(b_t[:], bvec[:])
    ones_t = const.tile([1, 128], BF16, tag="ones")
    nc.sync.dma_start(ones_t[:], ones_d[:])

    for g in range(1, min(LOOKAHEAD, ngroups)):
        pending.append(issue_loads(g))

    for g in range(ngroups):
        r = slice(g * GROUP, (g + 1) * GROUP)
        t0, t1, ts = pending.pop(0)
        if g + LOOKAHEAD < ngroups:
            pending.append(issue_loads(g + LOOKAHEAD))

        # Fold 32 neighbour slices to 2 with four in-place bf16 adds per
        # tensor (2x_1P DVE mode); the final level is finished on the PE by
        # PSUM accumulation so the serial tail after the last DMA is short.
        for t in (t0, t1):
            for lv in (16, 8, 4, 2):
                nc.vector.tensor_add(t[:, 0:lv * F], t[:, 0:lv * F],
                                     t[:, lv * F:2 * lv * F])

        # pacc[:, 0:128] = sum_n x_n0 (as [f, bh]), [:, 128:256] = sum_n x_n1,
        # [:, 256:384] = x_self.  Transposing matmuls: lhsT = data slice,
        # rhs = identity -> out[f, bh] = data[bh, f], accumulated over the
        # two remaining slices.
        pacc = ppool.tile([128, 3 * 128], F32, tag="pacc")
        nc.tensor.matmul(pacc[:, 256:384], ts[:], ident[:],
                         start=True, stop=True)
        nc.tensor.matmul(pacc[:, 0:128], t0[:, 0:F], ident[:],
                         start=True, stop=False)
        nc.tensor.matmul(pacc[:, 0:128], t0[:, F:2 * F], ident[:],
                         start=False, stop=True)
        nc.tensor.matmul(pacc[:, 128:256], t1[:, 0:F], ident[:],
                         start=True, stop=False)
        nc.tensor.matmul(pacc[:, 128:256], t1[:, F:2 * F], ident[:],
                         start=False, stop=True)

        # PSUM -> SBUF (bf16) in per-block copies so the self/t0 projections
        # don't wait for t1's transposes.
        sacc = spool.tile([128, 3 * 128], BF16, tag="sacc")
        nc.scalar.activation(sacc[:, 256:384], pacc[:, 256:384], COPY)
        nc.scalar.activation(sacc[:, 0:128], pacc[:, 0:128], COPY)
        nc.scalar.activation(sacc[:, 128:256], pacc[:, 128:256], COPY)

        # Projection: out[bh, d]; bias broadcast via K=1 matmuls.
        po = pout.tile([128, D], F32, tag="po")
        nc.tensor.matmul(po[:, 0:HALF], sacc[:, 256:384], wS_t[:],
                         start=True, stop=False)
        nc.tensor.matmul(po[:, 0:HALF], ones_t[:], b_t[:, 0:HALF],
                         start=False, stop=True)
        nc.tensor.matmul(po[:, HALF:D], sacc[:, 0:128], w0_t[:],
                         start=True, stop=False)
        nc.tensor.matmul(po[:, HALF:D], sacc[:, 128:256], w1_t[:],
                         start=False, stop=False)
        nc.tensor.matmul(po[:, HALF:D], ones_t[:], b_t[:, HALF:D],
                         start=False, stop=True)

        ob = opool.tile([128, D], F32, tag="ob")
        nc.scalar.activation(ob[:], po[:], RELU)
        nc.scalar.dma_start(out_d[r, :], ob[:])


def build_nc(ngroups=BH // GROUP):
    bh = ngroups * GROUP
    nc = bacc.Bacc("TRN2", target_bir_lowering=False, debug=False)
    xn0 = nc.dram_tensor("xn0", [bh, N * F], BF16, kind="ExternalInput")
    xn1 = nc.dram_tensor("xn1", [bh, N * F], BF16, kind="ExternalInput")
    xs = nc.dram_tensor("xs", [bh, F], BF16, kind="ExternalInput")
    w_s = nc.dram_tensor("w_s", [F, HALF], BF16, kind="ExternalInput")
    w0 = nc.dram_tensor("w0", [F, HALF], BF16, kind="ExternalInput")
    w1 = nc.dram_tensor("w1", [F, HALF], BF16, kind="ExternalInput")
    bvec = nc.dram_tensor("bvec", [1, D], BF16, kind="ExternalInput")
    ident_d = nc.dram_tensor("ident", [128, 128], BF16, kind="ExternalInput")
    ones_d = nc.dram_tensor("ones", [1, 128], BF16, kind="ExternalInput")
    out = nc.dram_tensor("out", [bh, D], F32, kind="ExternalOutput")

    ins = [t.ap() for t in (xn0, xn1, xs, w_s, w0, w1, bvec, ident_d, ones_d)]
    with nc.allow_low_precision("2e-2 rel-err budget admits bf16 datapath"):
        with tile.TileContext(nc) as tc:
            _tile_kernel(tc, [out.ap()], ins, ngroups)
    nc.compile()
    return nc


def make_in_maps(x_self, x_neigh_0, x_neigh_1, w_self, w_neigh_0, w_neigh_1, b):
    """Shard full inputs into per-core input maps (batch axis, 8 ways).

    All operands are cast to bf16 here (host side): the 2e-2 tolerance
    admits it and it halves the HBM traffic of this memory-bound kernel.
    """
    x_self = np.asarray(x_self, dtype=np.float32).astype(BF16NP)
    x_neigh_0 = np.asarray(x_neigh_0, dtype=np.float32).astype(BF16NP)
    x_neigh_1 = np.asarray(x_neigh_1, dtype=np.float32).astype(BF16NP)
    scale = np.float32(1.0 / (N * NR))
    w_s = np.asarray(w_self, dtype=np.float32).astype(BF16NP)
    w0 = (np.asarray(w_neigh_0, dtype=np.float32) * scale).astype(BF16NP)
    w1 = (np.asarray(w_neigh_1, dtype=np.float32) * scale).astype(BF16NP)
    bvec = np.asarray(b, dtype=np.float32).reshape(1, D).astype(BF16NP)
    ident = np.eye(128, dtype=np.float32).astype(BF16NP)
    ones = np.ones((1, 128), dtype=np.float32).astype(BF16NP)

    in_maps = []
    for c in range(NCORES):
        bs = slice(c * BSH, (c + 1) * BSH)
        in_maps.append({
            "xn0": np.ascontiguousarray(x_neigh_0[bs].reshape(BH, N * F)),
            "xn1": np.ascontiguousarray(x_neigh_1[bs].reshape(BH, N * F)),
            "xs": np.ascontiguousarray(x_self[bs].reshape(BH, F)),
            "w_s": w_s, "w0": w0, "w1": w1, "bvec": bvec,
            "ident": ident, "ones": ones,
        })
    return in_maps


_NC_CACHE = None


def kernel(x_self, x_neigh_0, x_neigh_1, w_self, w_neigh_0, w_neigh_1, b):
    global _NC_CACHE
    if _NC_CACHE is None:
        _NC_CACHE = build_nc()
    in_maps = make_in_maps(x_self, x_neigh_0, x_neigh_1,
                           w_self, w_neigh_0, w_neigh_1, b)
    res = bass_utils.run_bass_kernel_spmd(
        _NC_CACHE, in_maps, core_ids=list(range(NCORES)))
    out = np.concatenate([r["out"] for r in res.results], axis=0)
    return out.reshape(B, H, D)
